# revision 7
# baseline (speedup 1.0000x reference)
"""CosFace loss (N=2048, D=512, C=100000) on 8 Trainium2 NeuronCores.

Strategy (classifier/tensor parallel): shard the class dimension across the 8
cores (12500 classes each, padded to 12800). Each core streams its weight
shard once from HBM, computes cos = norm(emb) @ norm(w_shard).T in fp8e4
(DoubleRow, 2x PE rate) on the tensor engine, and reduces
sum_c exp(30*cos - 30) per batch row with the scalar engine's fused
exp+accumulate (fixed stabilizer 30 >= max logit since cos <= 1, so no max
pass). The embedding's l2-normalization is folded into the EXP's per-partition
scale (scale_n = 30 / ||e_n||), so the embedding path is just cast+transpose.
Weight rows are normalized on-device (sum-squares on GpSimd, rsqrt via ACT
Ln/Exp, multiply+cast on DVE), transposed d-major via PE in bf16, and cast to
fp8 during the PSUM->SBUF copy on DVE. The ground-truth logit for each row is
computed exactly in fp32 via an indirect-DMA gather of the 2048 target weight
rows on whichever core owns them. The host sums the per-core partial [2048]
vectors (disjoint class ranges) and applies the CosFace margin + logsumexp
formula in float64:

  lse_n = 30 + log(S_n - exp(30 c_n - 30) + exp(30 c_n - 12 - 30))
  nll_n = lse_n - (30 c_n - 12),  loss = mean_n nll_n

where S_n = sum_c exp(30 cos_nc - 30) (unmodified) and c_n = cos at the target
class. This is algebraically identical to softmax-CE with the margin one-hot.
"""

import numpy as np

# Problem geometry (hardcoded per contract).
N, D, C = 2048, 512, 100000
P = 128
N_CORES = 8
C_SHARD = C // N_CORES  # 12500
C_PAD = 12800  # padded shard size: 100 tiles of 128
NT = N // P  # 16 batch tiles
SCALE = 30.0
MARGIN = 0.4
STAB = 30.0  # logsumexp stabilizer; valid since cos <= 1
GROUP_COLS = 1536  # classes per PSUM accumulation group (3 banks)
MAX_SUB = GROUP_COLS // P

_CACHE = {}

# Debug knobs (bisecting hardware failures): set before first _build().
_BUILD_OPTS = {"gt": True, "ngroups": None, "fp8": True}


def _groups():
    # Small leading group so the first matmuls start early while the first
    # full-width weight DMA is still in flight.
    widths = [512]
    while sum(widths) < C_PAD:
        widths.append(min(GROUP_COLS, C_PAD - sum(widths)))
    gs = []
    c0 = 0
    for w in widths:
        real = max(0, min(C_SHARD - c0, w))
        gs.append((c0, w // P, w, real))
        c0 += w
    return gs


def _install_ntff_shim():
    """Register the axon NTFF profile hook if the image's antenv lacks it."""
    import sys
    import types

    try:
        from antenv.axon_hooks import get_axon_ntff_profile_hook  # noqa: F401

        return
    except ImportError:
        pass
    mod = types.ModuleType("antenv.axon_hooks")
    state = {"hook": None}
    mod.set_axon_ntff_profile_hook = lambda h: state.__setitem__("hook", h)
    mod.get_axon_ntff_profile_hook = lambda: state["hook"]
    sys.modules["antenv.axon_hooks"] = mod
    try:
        from trn_agent_boot.trn_boot import _ntff_profile_via_ctypes

        mod.set_axon_ntff_profile_hook(
            _ntff_profile_via_ctypes("/opt/axon/libaxon_pjrt.so")
        )
    except Exception:
        pass


def _build():
    if "nc" in _CACHE:
        return _CACHE["nc"]

    import concourse.bass as bass
    import concourse.tile as tile
    from concourse import bacc, mybir
    from concourse.masks import make_identity

    # Restrict the activation-table universe to the one set that contains
    # every function we use (Ln, Exp) so the compiler emits a single
    # ACT_TABLE_LOAD instead of thrashing between sets (~2.7us per switch).
    import concourse.hw_specs as hw_specs

    if not getattr(bacc, "_cosface_act_patch", False):
        _orig_get_tables = hw_specs.get_activation_tables

        def _one_set(arch):
            # act_func_set_id is positional, so keep every set in place and
            # instead remove Exp/Ln/Square from all other sets, forcing the
            # load-insertion pass to pick natural_log_exp_and_others for them.
            t = _orig_get_tables(arch)
            keep = {"Exp", "Ln", "Square"}
            return {
                name: (
                    funcs
                    if name == "natural_log_exp_and_others"
                    else {f for f in funcs if f.name not in keep}
                )
                for name, funcs in t.items()
            }

        bacc.get_activation_tables = _one_set
        bacc._cosface_act_patch = True

    f32 = mybir.dt.float32
    bf16 = mybir.dt.bfloat16
    i32 = mybir.dt.int32
    AF = mybir.ActivationFunctionType
    ALU = mybir.AluOpType
    AX = mybir.AxisListType
    use_fp8 = _BUILD_OPTS.get("fp8", False)
    mm_dt = mybir.dt.float8e4 if use_fp8 else bf16
    DR = mybir.MatmulPerfMode.DoubleRow

    groups = _groups()
    if _BUILD_OPTS.get("ngroups") is not None:
        groups = groups[: _BUILD_OPTS["ngroups"]]
    NG = len(groups)

    nc = bacc.Bacc(
        "TRN2", target_bir_lowering=False, debug=False, num_devices=N_CORES
    )
    w_d = nc.dram_tensor("w", [C_PAD, D], f32, kind="ExternalInput").ap()
    emb_d = nc.dram_tensor("emb", [N, D], f32, kind="ExternalInput").ap()
    gti_d = nc.dram_tensor("gt_idx", [P, NT], i32, kind="ExternalInput").ap()
    gtm_d = nc.dram_tensor("gt_mask", [P, NT], f32, kind="ExternalInput").ap()
    s_d = nc.dram_tensor("s_out", [P, NT], f32, kind="ExternalOutput").ap()
    g_d = nc.dram_tensor("g_out", [P, NT], f32, kind="ExternalOutput").ap()

    with tile.TileContext(nc) as tc:
        with (
            tc.tile_pool(name="persist", bufs=1) as persist,
            tc.tile_pool(name="wraw", bufs=3) as wraw_p,
            tc.tile_pool(name="wbf", bufs=2) as wbf_p,
            tc.tile_pool(name="wt", bufs=3) as wt_p,
            tc.tile_pool(name="stat", bufs=2) as stat_p,
            tc.tile_pool(name="gat", bufs=2) as gat_p,
            tc.tile_pool(name="dump", bufs=2) as dump_p,
            tc.tile_pool(name="pst", bufs=2, space="PSUM") as pst_p,
            tc.tile_pool(name="pbp", bufs=2, space="PSUM") as pb_p,
        ):
            # Transposes run in bf16 (fp8 PE transpose needs element-step-2
            # output); the psum->sbuf copy casts to mm_dt for the matmuls.
            tp_dt = bf16
            ident = persist.tile([P, P], tp_dt)
            make_identity(nc, ident[:])
            negstab = persist.tile([P, 1], f32)
            nc.vector.memset(negstab[:], -STAB)
            dumf = persist.tile([P, D], f32)  # DVE accum dummy
            dumg = persist.tile([P, D], f32)  # Pool accum dummy

            # ---- first weight group DMA up front (longest startup pole) ----
            c0_0, n_sub_0, width_0, _ = groups[0]
            wr0 = wraw_p.tile([P, MAX_SUB, D], f32, tag="wr")
            nc.sync.dma_start(
                wr0[:, :n_sub_0],
                w_d[c0_0 : c0_0 + width_0].rearrange("(s p) d -> p s d", p=P),
            )

            # ---- embedding: chunked load, cast, transpose; norms on Pool ----
            # l2-normalization of e is folded into the EXP scale (srse), so
            # the matmul path needs only cast+transpose of the raw rows.
            e_f = persist.tile([P, NT, D], f32)
            e_bf = persist.tile([P, NT, D], tp_dt)
            sse = persist.tile([P, NT], f32)
            e_T = persist.tile([P, 4, N], mm_dt)
            emb_r = emb_d.rearrange("(t p) d -> p t d", p=P)
            for q in range(4):
                nc.sync.dma_start(
                    e_f[:, 4 * q : 4 * (q + 1)], emb_r[:, 4 * q : 4 * (q + 1)]
                )
                for s in range(4):
                    t = 4 * q + s
                    nc.vector.tensor_copy(out=e_bf[:, t], in_=e_f[:, t])
                    nc.vector.scalar_tensor_tensor(
                        out=dumf[:],
                        in0=e_f[:, t],
                        scalar=1.0,
                        in1=e_f[:, t],
                        op0=ALU.mult,
                        op1=ALU.mult,
                        accum_out=sse[:, t : t + 1],
                    )
                for j in range(4):
                    ps = pst_p.tile([P, 4 * P], tp_dt, tag="pst")
                    for s in range(4):
                        t = 4 * q + s
                        nc.tensor.transpose(
                            ps[:, s * P : (s + 1) * P],
                            e_bf[:, t, j * P : (j + 1) * P],
                            ident[:],
                        )
                    nc.vector.tensor_copy(
                        out=e_T[:, j, q * 4 * P : (q + 1) * 4 * P], in_=ps[:]
                    )
            lne = persist.tile([P, NT], f32)
            rse = persist.tile([P, NT], f32)
            srse = persist.tile([P, NT], f32)
            nc.scalar.activation(lne[:], sse[:], AF.Ln)
            nc.scalar.activation(rse[:], lne[:], AF.Exp, scale=-0.5)
            nc.vector.tensor_scalar(
                out=srse[:], in0=rse[:], scalar1=SCALE, scalar2=None, op0=ALU.mult
            )

            # ---- ground-truth path (emitted mid-loop for overlap) ----
            def emit_gt():
                gti = persist.tile([P, NT], i32)
                nc.sync.dma_start(gti[:], gti_d)
                gtm = persist.tile([P, NT], f32)
                nc.sync.dma_start(gtm[:], gtm_d)
                dot = persist.tile([P, NT], f32)
                ssg = persist.tile([P, NT], f32)
                for t in range(NT):
                    wg = gat_p.tile([P, D], f32, tag="wg")
                    nc.gpsimd.indirect_dma_start(
                        out=wg[:],
                        out_offset=None,
                        in_=w_d,
                        in_offset=bass.IndirectOffsetOnAxis(
                            ap=gti[:, t : t + 1], axis=0
                        ),
                    )
                    nc.vector.scalar_tensor_tensor(
                        out=dumf[:],
                        in0=wg[:],
                        scalar=1.0,
                        in1=e_f[:, t],
                        op0=ALU.mult,
                        op1=ALU.mult,
                        accum_out=dot[:, t : t + 1],
                    )
                    nc.vector.scalar_tensor_tensor(
                        out=dumf[:],
                        in0=wg[:],
                        scalar=1.0,
                        in1=wg[:],
                        op0=ALU.mult,
                        op1=ALU.mult,
                        accum_out=ssg[:, t : t + 1],
                    )
                lng = persist.tile([P, NT], f32)
                rsg = persist.tile([P, NT], f32)
                nc.scalar.activation(lng[:], ssg[:], AF.Ln)
                nc.scalar.activation(rsg[:], lng[:], AF.Exp, scale=-0.5)
                gtc = persist.tile([P, NT], f32)
                nc.vector.tensor_tensor(
                    out=gtc[:], in0=dot[:], in1=rsg[:], op=ALU.mult
                )
                nc.vector.tensor_tensor(
                    out=gtc[:], in0=gtc[:], in1=rse[:], op=ALU.mult
                )
                nc.vector.tensor_tensor(
                    out=gtc[:], in0=gtc[:], in1=gtm[:], op=ALU.mult
                )
                nc.sync.dma_start(g_d, gtc[:])

            # ---- main streaming loop over class groups ----
            sexp = persist.tile([P, NT * NG], f32)
            for gi, (c0, n_sub, width, real) in enumerate(groups):
                if gi == 0:
                    wr = wr0
                else:
                    wr = wraw_p.tile([P, MAX_SUB, D], f32, tag="wr")
                    nc.sync.dma_start(
                        wr[:, :n_sub],
                        w_d[c0 : c0 + width].rearrange("(s p) d -> p s d", p=P),
                    )
                ssw = stat_p.tile([P, MAX_SUB], f32, tag="ssw")
                for s in range(n_sub):
                    nc.vector.scalar_tensor_tensor(
                        out=dumf[:],
                        in0=wr[:, s],
                        scalar=1.0,
                        in1=wr[:, s],
                        op0=ALU.mult,
                        op1=ALU.mult,
                        accum_out=ssw[:, s : s + 1],
                    )
                lnw = stat_p.tile([P, MAX_SUB], f32, tag="lnw")
                rsw = stat_p.tile([P, MAX_SUB], f32, tag="rsw")
                nc.scalar.activation(lnw[:, :n_sub], ssw[:, :n_sub], AF.Ln)
                nc.scalar.activation(
                    rsw[:, :n_sub], lnw[:, :n_sub], AF.Exp, scale=-0.5
                )
                wb = wbf_p.tile([P, MAX_SUB, D], tp_dt, tag="wb")
                for s in range(n_sub):
                    nc.gpsimd.tensor_scalar(
                        out=wb[:, s],
                        in0=wr[:, s],
                        scalar1=rsw[:, s : s + 1],
                        scalar2=None,
                        op0=ALU.mult,
                    )
                # transpose to [d, c] layout (bf16), cast to fp8 in the copy
                wt = wt_p.tile([P, 4, GROUP_COLS], mm_dt, tag="wt")
                for j in range(4):
                    for qq in range((n_sub + 3) // 4):
                        ps = pst_p.tile([P, 4 * P], tp_dt, tag="pst")
                        hi = min(4, n_sub - qq * 4)
                        for s2 in range(hi):
                            s = qq * 4 + s2
                            nc.tensor.transpose(
                                ps[:, s2 * P : (s2 + 1) * P],
                                wb[:, s, j * P : (j + 1) * P],
                                ident[:],
                            )
                        nc.vector.tensor_copy(
                            out=wt[:, j, qq * 4 * P : qq * 4 * P + hi * P],
                            in_=ps[:, : hi * P],
                        )
                n_chunks = width // 512
                for t in range(NT):
                    pb = pb_p.tile([P, GROUP_COLS], f32, tag="pb")
                    if use_fp8:
                        for j in range(2):
                            for cc in range(n_chunks):
                                nc.tensor.matmul(
                                    pb[:, cc * 512 : (cc + 1) * 512],
                                    lhsT=e_T[
                                        :, 2 * j : 2 * j + 2, t * P : (t + 1) * P
                                    ],
                                    rhs=wt[
                                        :,
                                        2 * j : 2 * j + 2,
                                        cc * 512 : (cc + 1) * 512,
                                    ],
                                    start=(j == 0),
                                    stop=(j == 1),
                                    perf_mode=DR,
                                )
                    else:
                        for j in range(4):
                            for cc in range(n_chunks):
                                nc.tensor.matmul(
                                    pb[:, cc * 512 : (cc + 1) * 512],
                                    lhsT=e_T[:, j, t * P : (t + 1) * P],
                                    rhs=wt[:, j, cc * 512 : (cc + 1) * 512],
                                    start=(j == 0),
                                    stop=(j == 3),
                                )
                    du = dump_p.tile([P, GROUP_COLS], bf16, tag="du")
                    nc.scalar.activation(
                        du[:, :real],
                        pb[:, :real],
                        AF.Exp,
                        scale=srse[:, t : t + 1],
                        bias=negstab[:, :1],
                        accum_out=sexp[:, t * NG + gi : t * NG + gi + 1],
                    )
                if gi == 4 and _BUILD_OPTS.get("gt", True):
                    emit_gt()
                if gi == len(groups) - 1 and len(groups) <= 4 and _BUILD_OPTS.get("gt", True):
                    emit_gt()

            spart = persist.tile([P, NT], f32)
            for t in range(NT):
                nc.vector.tensor_reduce(
                    spart[:, t : t + 1],
                    sexp[:, t * NG : (t + 1) * NG],
                    AX.X,
                    ALU.add,
                )
            nc.sync.dma_start(s_d, spart[:])

    nc.compile()
    _CACHE["nc"] = nc
    return nc


def run(embedding, ground_truth, weight, trace=False):
    """Run the sharded device kernel; returns (loss_scalar, BassKernelResults)."""
    import concourse.bass_utils as bass_utils

    if trace:
        _install_ntff_shim()

    nc = _build()

    emb = np.ascontiguousarray(np.asarray(embedding, dtype=np.float32))
    w_full = np.ascontiguousarray(np.asarray(weight, dtype=np.float32))
    gt = np.asarray(ground_truth).astype(np.int64)

    in_maps = []
    for k in range(N_CORES):
        lo = k * C_SHARD
        wshard = np.empty((C_PAD, D), dtype=np.float32)
        wshard[:C_SHARD] = w_full[lo : lo + C_SHARD]
        wshard[C_SHARD:] = 1.0  # pad rows; excluded from the exp reduction
        loc = gt - lo
        mask = (loc >= 0) & (loc < C_SHARD)
        idx = np.clip(loc, 0, C_SHARD - 1).astype(np.int32)
        in_maps.append(
            {
                "w": wshard,
                "emb": emb,
                "gt_idx": np.ascontiguousarray(idx.reshape(NT, P).T),
                "gt_mask": np.ascontiguousarray(
                    mask.reshape(NT, P).T.astype(np.float32)
                ),
            }
        )

    kwargs = {}
    if trace:
        import os

        os.environ["BASS_PERFETTO_PROFILE_ALL_CORES"] = "1"
        kwargs = dict(trace=True, trace_cores=list(range(N_CORES)), stitch_traces=False)

    res = bass_utils.run_bass_kernel_spmd(
        nc, in_maps, core_ids=list(range(N_CORES)), **kwargs
    )

    S = np.zeros(N, dtype=np.float64)
    cg = np.zeros(N, dtype=np.float64)
    for k in range(N_CORES):
        S += res.results[k]["s_out"].astype(np.float64).T.reshape(N)
        cg += res.results[k]["g_out"].astype(np.float64).T.reshape(N)

    lse = STAB + np.log(
        S - np.exp(SCALE * cg - STAB) + np.exp(SCALE * cg - SCALE * MARGIN - STAB)
    )
    nll = lse - (SCALE * cg - SCALE * MARGIN)
    loss = np.float32(nll.mean())
    return loss, res


def kernel(embedding, ground_truth, weight):
    loss, _ = run(embedding, ground_truth, weight, trace=False)
    return np.asarray(loss, dtype=np.float32)


# revision 8
# speedup vs baseline: 2.7379x; 2.7379x over previous
"""CosFace loss (N=2048, D=512, C=100000) on 8 Trainium2 NeuronCores.

Strategy (classifier/tensor parallel): shard the class dimension across the 8
cores (12500 classes each, padded to 12800). Each core streams its weight
shard once from HBM, computes cos = norm(emb) @ norm(w_shard).T in fp8e4
(DoubleRow, 2x PE rate) on the tensor engine, and reduces
sum_c exp(30*cos - 30) per batch row with the scalar engine's fused
exp+accumulate (fixed stabilizer 30 >= max logit since cos <= 1, so no max
pass). The embedding's l2-normalization is folded into the EXP's per-partition
scale (scale_n = 30 / ||e_n||), so the embedding path is just cast+transpose.
Weight rows are normalized on-device (sum-squares on GpSimd, rsqrt via ACT
Ln/Exp, multiply+cast on DVE), transposed d-major via PE in bf16, and cast to
fp8 during the PSUM->SBUF copy on DVE. The ground-truth logit for each row is
computed exactly in fp32 via an indirect-DMA gather of the 2048 target weight
rows on whichever core owns them. The host sums the per-core partial [2048]
vectors (disjoint class ranges) and applies the CosFace margin + logsumexp
formula in float64:

  lse_n = 30 + log(S_n - exp(30 c_n - 30) + exp(30 c_n - 12 - 30))
  nll_n = lse_n - (30 c_n - 12),  loss = mean_n nll_n

where S_n = sum_c exp(30 cos_nc - 30) (unmodified) and c_n = cos at the target
class. This is algebraically identical to softmax-CE with the margin one-hot.
"""

import numpy as np

# Problem geometry (hardcoded per contract).
N, D, C = 2048, 512, 100000
P = 128
N_CORES = 8
C_SHARD = C // N_CORES  # 12500
C_PAD = 12800  # padded shard size: 100 tiles of 128
NT = N // P  # 16 batch tiles
SCALE = 30.0
MARGIN = 0.4
STAB = 30.0  # logsumexp stabilizer; valid since cos <= 1
GROUP_COLS = 1536  # classes per PSUM accumulation group (3 banks)
MAX_SUB = GROUP_COLS // P

_CACHE = {}

# Debug knobs (bisecting hardware failures): set before first _build().
_BUILD_OPTS = {"gt": True, "ngroups": None, "fp8": True}


def _groups():
    # Small leading group so the first matmuls start early while the first
    # full-width weight DMA is still in flight.
    widths = [512]
    while sum(widths) < C_PAD:
        widths.append(min(GROUP_COLS, C_PAD - sum(widths)))
    gs = []
    c0 = 0
    for w in widths:
        real = max(0, min(C_SHARD - c0, w))
        gs.append((c0, w // P, w, real))
        c0 += w
    return gs


def _install_ntff_shim():
    """Register the axon NTFF profile hook if the image's antenv lacks it."""
    import sys
    import types

    try:
        from antenv.axon_hooks import get_axon_ntff_profile_hook  # noqa: F401

        return
    except ImportError:
        pass
    mod = types.ModuleType("antenv.axon_hooks")
    state = {"hook": None}
    mod.set_axon_ntff_profile_hook = lambda h: state.__setitem__("hook", h)
    mod.get_axon_ntff_profile_hook = lambda: state["hook"]
    sys.modules["antenv.axon_hooks"] = mod
    try:
        from trn_agent_boot.trn_boot import _ntff_profile_via_ctypes

        mod.set_axon_ntff_profile_hook(
            _ntff_profile_via_ctypes("/opt/axon/libaxon_pjrt.so")
        )
    except Exception:
        pass


def _build():
    if "nc" in _CACHE:
        return _CACHE["nc"]

    import concourse.bass as bass
    import concourse.tile as tile
    from concourse import bacc, mybir
    from concourse.masks import make_identity

    # Restrict the activation-table universe to the one set that contains
    # every function we use (Ln, Exp) so the compiler emits a single
    # ACT_TABLE_LOAD instead of thrashing between sets (~2.7us per switch).
    import concourse.hw_specs as hw_specs

    if not getattr(bacc, "_cosface_act_patch", False):
        _orig_get_tables = hw_specs.get_activation_tables

        def _one_set(arch):
            # act_func_set_id is positional, so keep every set in place and
            # instead remove Exp/Ln/Square from all other sets, forcing the
            # load-insertion pass to pick natural_log_exp_and_others for them.
            t = _orig_get_tables(arch)
            keep = {"Exp", "Ln", "Square"}
            return {
                name: (
                    funcs
                    if name == "natural_log_exp_and_others"
                    else {f for f in funcs if f.name not in keep}
                )
                for name, funcs in t.items()
            }

        bacc.get_activation_tables = _one_set
        bacc._cosface_act_patch = True

    f32 = mybir.dt.float32
    bf16 = mybir.dt.bfloat16
    i32 = mybir.dt.int32
    AF = mybir.ActivationFunctionType
    ALU = mybir.AluOpType
    AX = mybir.AxisListType
    use_fp8 = _BUILD_OPTS.get("fp8", False)
    mm_dt = mybir.dt.float8e4 if use_fp8 else bf16
    DR = mybir.MatmulPerfMode.DoubleRow

    groups = _groups()
    if _BUILD_OPTS.get("ngroups") is not None:
        groups = groups[: _BUILD_OPTS["ngroups"]]
    NG = len(groups)

    nc = bacc.Bacc(
        "TRN2", target_bir_lowering=False, debug=False, num_devices=N_CORES
    )
    w_d = nc.dram_tensor("w", [C_PAD, D], f32, kind="ExternalInput").ap()
    emb_d = nc.dram_tensor("emb", [N, D], f32, kind="ExternalInput").ap()
    gti_d = nc.dram_tensor("gt_idx", [P, NT], i32, kind="ExternalInput").ap()
    gtm_d = nc.dram_tensor("gt_mask", [P, NT], f32, kind="ExternalInput").ap()
    s_d = nc.dram_tensor("s_out", [P, NT], f32, kind="ExternalOutput").ap()
    g_d = nc.dram_tensor("g_out", [P, NT], f32, kind="ExternalOutput").ap()

    with tile.TileContext(nc) as tc:
        with (
            tc.tile_pool(name="persist", bufs=1) as persist,
            tc.tile_pool(name="wraw", bufs=3) as wraw_p,
            tc.tile_pool(name="wbf", bufs=2) as wbf_p,
            tc.tile_pool(name="wt", bufs=3) as wt_p,
            tc.tile_pool(name="stat", bufs=2) as stat_p,
            tc.tile_pool(name="gat", bufs=2) as gat_p,
            tc.tile_pool(name="dump", bufs=2) as dump_p,
            tc.tile_pool(name="pst", bufs=2, space="PSUM") as pst_p,
            tc.tile_pool(name="pbp", bufs=2, space="PSUM") as pb_p,
        ):
            # Transposes run in bf16 (fp8 PE transpose needs element-step-2
            # output); the psum->sbuf copy casts to mm_dt for the matmuls.
            tp_dt = bf16
            ident = persist.tile([P, P], tp_dt)
            make_identity(nc, ident[:])
            negstab = persist.tile([P, 1], f32)
            nc.vector.memset(negstab[:], -STAB)
            dumf = persist.tile([P, D], f32)  # DVE accum dummy
            dumg = persist.tile([P, D], f32)  # Pool accum dummy

            # ---- first weight group DMA up front (longest startup pole) ----
            c0_0, n_sub_0, width_0, _ = groups[0]
            wr0 = wraw_p.tile([P, MAX_SUB, D], f32, tag="wr")
            nc.sync.dma_start(
                wr0[:, :n_sub_0],
                w_d[c0_0 : c0_0 + width_0].rearrange("(s p) d -> p s d", p=P),
            )

            # ---- embedding: chunked load, cast, transpose; norms on Pool ----
            # l2-normalization of e is folded into the EXP scale (srse), so
            # the matmul path needs only cast+transpose of the raw rows.
            e_f = persist.tile([P, NT, D], f32)
            e_bf = persist.tile([P, NT, D], tp_dt)
            sse = persist.tile([P, NT], f32)
            e_T = persist.tile([P, 4, N], mm_dt)
            emb_r = emb_d.rearrange("(t p) d -> p t d", p=P)
            for q in range(4):
                nc.sync.dma_start(
                    e_f[:, 4 * q : 4 * (q + 1)], emb_r[:, 4 * q : 4 * (q + 1)]
                )
                for s in range(4):
                    t = 4 * q + s
                    nc.vector.tensor_copy(out=e_bf[:, t], in_=e_f[:, t])
                    nc.vector.scalar_tensor_tensor(
                        out=dumf[:],
                        in0=e_f[:, t],
                        scalar=1.0,
                        in1=e_f[:, t],
                        op0=ALU.mult,
                        op1=ALU.mult,
                        accum_out=sse[:, t : t + 1],
                    )
                for j in range(4):
                    ps = pst_p.tile([P, 4 * P], tp_dt, tag="pst")
                    for s in range(4):
                        t = 4 * q + s
                        nc.tensor.transpose(
                            ps[:, s * P : (s + 1) * P],
                            e_bf[:, t, j * P : (j + 1) * P],
                            ident[:],
                        )
                    nc.vector.tensor_copy(
                        out=e_T[:, j, q * 4 * P : (q + 1) * 4 * P], in_=ps[:]
                    )
            lne = persist.tile([P, NT], f32)
            rse = persist.tile([P, NT], f32)
            srse = persist.tile([P, NT], f32)
            nc.scalar.activation(lne[:], sse[:], AF.Ln)
            nc.scalar.activation(rse[:], lne[:], AF.Exp, scale=-0.5)
            nc.vector.tensor_scalar(
                out=srse[:], in0=rse[:], scalar1=SCALE, scalar2=None, op0=ALU.mult
            )

            # ---- ground-truth path (emitted mid-loop for overlap) ----
            def emit_gt():
                gti = persist.tile([P, NT], i32)
                nc.sync.dma_start(gti[:], gti_d)
                gtm = persist.tile([P, NT], f32)
                nc.sync.dma_start(gtm[:], gtm_d)
                dot = persist.tile([P, NT], f32)
                ssg = persist.tile([P, NT], f32)
                for t in range(NT):
                    wg = gat_p.tile([P, D], f32, tag="wg")
                    nc.gpsimd.indirect_dma_start(
                        out=wg[:],
                        out_offset=None,
                        in_=w_d,
                        in_offset=bass.IndirectOffsetOnAxis(
                            ap=gti[:, t : t + 1], axis=0
                        ),
                    )
                    nc.vector.scalar_tensor_tensor(
                        out=dumf[:],
                        in0=wg[:],
                        scalar=1.0,
                        in1=e_f[:, t],
                        op0=ALU.mult,
                        op1=ALU.mult,
                        accum_out=dot[:, t : t + 1],
                    )
                    nc.vector.scalar_tensor_tensor(
                        out=dumf[:],
                        in0=wg[:],
                        scalar=1.0,
                        in1=wg[:],
                        op0=ALU.mult,
                        op1=ALU.mult,
                        accum_out=ssg[:, t : t + 1],
                    )
                lng = persist.tile([P, NT], f32)
                rsg = persist.tile([P, NT], f32)
                nc.scalar.activation(lng[:], ssg[:], AF.Ln)
                nc.scalar.activation(rsg[:], lng[:], AF.Exp, scale=-0.5)
                gtc = persist.tile([P, NT], f32)
                nc.vector.tensor_tensor(
                    out=gtc[:], in0=dot[:], in1=rsg[:], op=ALU.mult
                )
                nc.vector.tensor_tensor(
                    out=gtc[:], in0=gtc[:], in1=rse[:], op=ALU.mult
                )
                nc.vector.tensor_tensor(
                    out=gtc[:], in0=gtc[:], in1=gtm[:], op=ALU.mult
                )
                nc.sync.dma_start(g_d, gtc[:])

            # ---- main streaming loop over class groups ----
            sexp = persist.tile([P, NT * NG], f32)
            for gi, (c0, n_sub, width, real) in enumerate(groups):
                if gi == 0:
                    wr = wr0
                else:
                    wr = wraw_p.tile([P, MAX_SUB, D], f32, tag="wr")
                    nc.sync.dma_start(
                        wr[:, :n_sub],
                        w_d[c0 : c0 + width].rearrange("(s p) d -> p s d", p=P),
                    )
                ssw = stat_p.tile([P, MAX_SUB], f32, tag="ssw")
                for s in range(n_sub):
                    nc.vector.scalar_tensor_tensor(
                        out=dumf[:],
                        in0=wr[:, s],
                        scalar=1.0,
                        in1=wr[:, s],
                        op0=ALU.mult,
                        op1=ALU.mult,
                        accum_out=ssw[:, s : s + 1],
                    )
                lnw = stat_p.tile([P, MAX_SUB], f32, tag="lnw")
                rsw = stat_p.tile([P, MAX_SUB], f32, tag="rsw")
                nc.scalar.activation(lnw[:, :n_sub], ssw[:, :n_sub], AF.Ln)
                nc.scalar.activation(
                    rsw[:, :n_sub], lnw[:, :n_sub], AF.Exp, scale=-0.5
                )
                wb = wbf_p.tile([P, MAX_SUB, D], tp_dt, tag="wb")
                for s in range(n_sub):
                    nc.vector.tensor_scalar(
                        out=wb[:, s],
                        in0=wr[:, s],
                        scalar1=rsw[:, s : s + 1],
                        scalar2=None,
                        op0=ALU.mult,
                    )
                # transpose to [d, c] layout (bf16), cast to fp8 in the copy
                wt = wt_p.tile([P, 4, GROUP_COLS], mm_dt, tag="wt")
                for j in range(4):
                    for qq in range((n_sub + 3) // 4):
                        ps = pst_p.tile([P, 4 * P], tp_dt, tag="pst")
                        hi = min(4, n_sub - qq * 4)
                        for s2 in range(hi):
                            s = qq * 4 + s2
                            nc.tensor.transpose(
                                ps[:, s2 * P : (s2 + 1) * P],
                                wb[:, s, j * P : (j + 1) * P],
                                ident[:],
                            )
                        nc.vector.tensor_copy(
                            out=wt[:, j, qq * 4 * P : qq * 4 * P + hi * P],
                            in_=ps[:, : hi * P],
                        )
                n_chunks = width // 512
                for t in range(NT):
                    pb = pb_p.tile([P, GROUP_COLS], f32, tag="pb")
                    if use_fp8:
                        for j in range(2):
                            for cc in range(n_chunks):
                                nc.tensor.matmul(
                                    pb[:, cc * 512 : (cc + 1) * 512],
                                    lhsT=e_T[
                                        :, 2 * j : 2 * j + 2, t * P : (t + 1) * P
                                    ],
                                    rhs=wt[
                                        :,
                                        2 * j : 2 * j + 2,
                                        cc * 512 : (cc + 1) * 512,
                                    ],
                                    start=(j == 0),
                                    stop=(j == 1),
                                    perf_mode=DR,
                                )
                    else:
                        for j in range(4):
                            for cc in range(n_chunks):
                                nc.tensor.matmul(
                                    pb[:, cc * 512 : (cc + 1) * 512],
                                    lhsT=e_T[:, j, t * P : (t + 1) * P],
                                    rhs=wt[:, j, cc * 512 : (cc + 1) * 512],
                                    start=(j == 0),
                                    stop=(j == 3),
                                )
                    du = dump_p.tile([P, GROUP_COLS], bf16, tag="du")
                    nc.scalar.activation(
                        du[:, :real],
                        pb[:, :real],
                        AF.Exp,
                        scale=srse[:, t : t + 1],
                        bias=negstab[:, :1],
                        accum_out=sexp[:, t * NG + gi : t * NG + gi + 1],
                    )
                if gi == 4 and _BUILD_OPTS.get("gt", True):
                    emit_gt()
                if gi == len(groups) - 1 and len(groups) <= 4 and _BUILD_OPTS.get("gt", True):
                    emit_gt()

            spart = persist.tile([P, NT], f32)
            for t in range(NT):
                nc.vector.tensor_reduce(
                    spart[:, t : t + 1],
                    sexp[:, t * NG : (t + 1) * NG],
                    AX.X,
                    ALU.add,
                )
            nc.sync.dma_start(s_d, spart[:])

    nc.compile()
    _CACHE["nc"] = nc
    return nc


def run(embedding, ground_truth, weight, trace=False):
    """Run the sharded device kernel; returns (loss_scalar, BassKernelResults)."""
    import concourse.bass_utils as bass_utils

    if trace:
        _install_ntff_shim()

    nc = _build()

    emb = np.ascontiguousarray(np.asarray(embedding, dtype=np.float32))
    w_full = np.ascontiguousarray(np.asarray(weight, dtype=np.float32))
    gt = np.asarray(ground_truth).astype(np.int64)

    in_maps = []
    for k in range(N_CORES):
        lo = k * C_SHARD
        wshard = np.empty((C_PAD, D), dtype=np.float32)
        wshard[:C_SHARD] = w_full[lo : lo + C_SHARD]
        wshard[C_SHARD:] = 1.0  # pad rows; excluded from the exp reduction
        loc = gt - lo
        mask = (loc >= 0) & (loc < C_SHARD)
        idx = np.clip(loc, 0, C_SHARD - 1).astype(np.int32)
        in_maps.append(
            {
                "w": wshard,
                "emb": emb,
                "gt_idx": np.ascontiguousarray(idx.reshape(NT, P).T),
                "gt_mask": np.ascontiguousarray(
                    mask.reshape(NT, P).T.astype(np.float32)
                ),
            }
        )

    kwargs = {}
    if trace:
        import os

        os.environ["BASS_PERFETTO_PROFILE_ALL_CORES"] = "1"
        kwargs = dict(trace=True, trace_cores=list(range(N_CORES)), stitch_traces=False)

    res = bass_utils.run_bass_kernel_spmd(
        nc, in_maps, core_ids=list(range(N_CORES)), **kwargs
    )

    S = np.zeros(N, dtype=np.float64)
    cg = np.zeros(N, dtype=np.float64)
    for k in range(N_CORES):
        S += res.results[k]["s_out"].astype(np.float64).T.reshape(N)
        cg += res.results[k]["g_out"].astype(np.float64).T.reshape(N)

    lse = STAB + np.log(
        S - np.exp(SCALE * cg - STAB) + np.exp(SCALE * cg - SCALE * MARGIN - STAB)
    )
    nll = lse - (SCALE * cg - SCALE * MARGIN)
    loss = np.float32(nll.mean())
    return loss, res


def kernel(embedding, ground_truth, weight):
    loss, _ = run(embedding, ground_truth, weight, trace=False)
    return np.asarray(loss, dtype=np.float32)


# revision 10
# speedup vs baseline: 2.8245x; 1.0316x over previous
"""CosFace loss (N=2048, D=512, C=100000) on 8 Trainium2 NeuronCores.

Strategy (classifier/tensor parallel): shard the class dimension across the 8
cores (12500 classes each, padded to 12800). Each core streams its weight
shard once from HBM, computes cos = norm(emb) @ norm(w_shard).T in fp8e4
(DoubleRow, 2x PE rate) on the tensor engine, and reduces
sum_c exp(30*cos - 30) per batch row with the scalar engine's fused
exp+accumulate (fixed stabilizer 30 >= max logit since cos <= 1, so no max
pass). The embedding's l2-normalization is folded into the EXP's per-partition
scale (scale_n = 30 / ||e_n||), so the embedding path is just cast+transpose.
Weight rows are normalized on-device (sum-squares on GpSimd, rsqrt via ACT
Ln/Exp, multiply+cast on DVE), transposed d-major via PE in bf16, and cast to
fp8 during the PSUM->SBUF copy on DVE. The ground-truth logit for each row is
computed exactly in fp32 via an indirect-DMA gather of the 2048 target weight
rows on whichever core owns them. The host sums the per-core partial [2048]
vectors (disjoint class ranges) and applies the CosFace margin + logsumexp
formula in float64:

  lse_n = 30 + log(S_n - exp(30 c_n - 30) + exp(30 c_n - 12 - 30))
  nll_n = lse_n - (30 c_n - 12),  loss = mean_n nll_n

where S_n = sum_c exp(30 cos_nc - 30) (unmodified) and c_n = cos at the target
class. This is algebraically identical to softmax-CE with the margin one-hot.
"""

import numpy as np

# Problem geometry (hardcoded per contract).
N, D, C = 2048, 512, 100000
P = 128
N_CORES = 8
C_SHARD = C // N_CORES  # 12500
C_PAD = 12800  # padded shard size: 100 tiles of 128
NT = N // P  # 16 batch tiles
SCALE = 30.0
MARGIN = 0.4
STAB = 30.0  # logsumexp stabilizer; valid since cos <= 1
GROUP_COLS = 1536  # classes per PSUM accumulation group (3 banks)
MAX_SUB = GROUP_COLS // P

_CACHE = {}

# Debug knobs (bisecting hardware failures): set before first _build().
_BUILD_OPTS = {"gt": True, "ngroups": None, "fp8": True}


def _groups():
    # Processing order: the mostly-pad tail region (real=212) first -- its
    # small width makes the first matmul+EXP start early -- then the seven
    # full 1536 groups, then 1024/512 so the pipeline tail drains fast.
    plan = [(12288, 512), (0, 1536), (1536, 1536), (3072, 1536), (4608, 1536),
            (6144, 1536), (7680, 1536), (9216, 1536), (10752, 1024),
            (11776, 512)]
    gs = []
    for c0, w in plan:
        real = max(0, min(C_SHARD - c0, w))
        gs.append((c0, w // P, w, real))
    return gs


def _install_ntff_shim():
    """Register the axon NTFF profile hook if the image's antenv lacks it."""
    import sys
    import types

    try:
        from antenv.axon_hooks import get_axon_ntff_profile_hook  # noqa: F401

        return
    except ImportError:
        pass
    mod = types.ModuleType("antenv.axon_hooks")
    state = {"hook": None}
    mod.set_axon_ntff_profile_hook = lambda h: state.__setitem__("hook", h)
    mod.get_axon_ntff_profile_hook = lambda: state["hook"]
    sys.modules["antenv.axon_hooks"] = mod
    try:
        from trn_agent_boot.trn_boot import _ntff_profile_via_ctypes

        mod.set_axon_ntff_profile_hook(
            _ntff_profile_via_ctypes("/opt/axon/libaxon_pjrt.so")
        )
    except Exception:
        pass


def _build():
    if "nc" in _CACHE:
        return _CACHE["nc"]

    import concourse.bass as bass
    import concourse.tile as tile
    from concourse import bacc, mybir
    from concourse.masks import make_identity

    # Restrict the activation-table universe to the one set that contains
    # every function we use (Ln, Exp) so the compiler emits a single
    # ACT_TABLE_LOAD instead of thrashing between sets (~2.7us per switch).
    import concourse.hw_specs as hw_specs

    if not getattr(bacc, "_cosface_act_patch", False):
        _orig_get_tables = hw_specs.get_activation_tables

        def _one_set(arch):
            # act_func_set_id is positional, so keep every set in place and
            # instead remove Exp/Ln/Square from all other sets, forcing the
            # load-insertion pass to pick natural_log_exp_and_others for them.
            t = _orig_get_tables(arch)
            keep = {"Exp", "Ln", "Square"}
            return {
                name: (
                    funcs
                    if name == "natural_log_exp_and_others"
                    else {f for f in funcs if f.name not in keep}
                )
                for name, funcs in t.items()
            }

        bacc.get_activation_tables = _one_set
        bacc._cosface_act_patch = True

    f32 = mybir.dt.float32
    bf16 = mybir.dt.bfloat16
    i32 = mybir.dt.int32
    AF = mybir.ActivationFunctionType
    ALU = mybir.AluOpType
    AX = mybir.AxisListType
    use_fp8 = _BUILD_OPTS.get("fp8", False)
    mm_dt = mybir.dt.float8e4 if use_fp8 else bf16
    DR = mybir.MatmulPerfMode.DoubleRow

    groups = _groups()
    if _BUILD_OPTS.get("ngroups") is not None:
        groups = groups[: _BUILD_OPTS["ngroups"]]
    NG = len(groups)

    nc = bacc.Bacc(
        "TRN2", target_bir_lowering=False, debug=False, num_devices=N_CORES
    )
    w_d = nc.dram_tensor("w", [C_PAD, D], f32, kind="ExternalInput").ap()
    emb_d = nc.dram_tensor("emb", [N, D], f32, kind="ExternalInput").ap()
    gti_d = nc.dram_tensor("gt_idx", [P, NT], i32, kind="ExternalInput").ap()
    gtm_d = nc.dram_tensor("gt_mask", [P, NT], f32, kind="ExternalInput").ap()
    s_d = nc.dram_tensor("s_out", [P, NT], f32, kind="ExternalOutput").ap()
    g_d = nc.dram_tensor("g_out", [P, NT], f32, kind="ExternalOutput").ap()

    with tile.TileContext(nc) as tc:
        with (
            tc.tile_pool(name="persist", bufs=1) as persist,
            tc.tile_pool(name="wraw", bufs=3) as wraw_p,
            tc.tile_pool(name="wbf", bufs=3) as wbf_p,
            tc.tile_pool(name="wt", bufs=3) as wt_p,
            tc.tile_pool(name="stat", bufs=2) as stat_p,
            tc.tile_pool(name="gat", bufs=2) as gat_p,
            tc.tile_pool(name="dump", bufs=2) as dump_p,
            tc.tile_pool(name="pst", bufs=2, space="PSUM") as pst_p,
            tc.tile_pool(name="pbp", bufs=2, space="PSUM") as pb_p,
        ):
            # Transposes run in bf16 (fp8 PE transpose needs element-step-2
            # output); the psum->sbuf copy casts to mm_dt for the matmuls.
            tp_dt = bf16
            ident = persist.tile([P, P], tp_dt)
            make_identity(nc, ident[:])
            negstab = persist.tile([P, 1], f32)
            nc.vector.memset(negstab[:], -STAB)
            dumf = persist.tile([P, D], f32)  # DVE accum dummy
            dumg = persist.tile([P, D], f32)  # Pool accum dummy

            # ---- first weight group DMA up front (longest startup pole) ----
            c0_0, n_sub_0, width_0, _ = groups[0]
            wr0 = wraw_p.tile([P, MAX_SUB, D], f32, tag="wr")
            nc.sync.dma_start(
                wr0[:, :n_sub_0],
                w_d[c0_0 : c0_0 + width_0].rearrange("(s p) d -> p s d", p=P),
            )

            # ---- embedding: chunked load, cast, transpose; norms on Pool ----
            # l2-normalization of e is folded into the EXP scale (srse), so
            # the matmul path needs only cast+transpose of the raw rows.
            e_f = persist.tile([P, NT, D], f32)
            e_bf = persist.tile([P, NT, D], tp_dt)
            sse = persist.tile([P, NT], f32)
            e_T = persist.tile([P, 4, N], mm_dt)
            lne = persist.tile([P, NT], f32)
            rse = persist.tile([P, NT], f32)
            srse = persist.tile([P, NT], f32)
            emb_r = emb_d.rearrange("(t p) d -> p t d", p=P)
            for q in range(4):
                nc.sync.dma_start(
                    e_f[:, 4 * q : 4 * (q + 1)], emb_r[:, 4 * q : 4 * (q + 1)]
                )
                for s in range(4):
                    t = 4 * q + s
                    nc.vector.tensor_copy(out=e_bf[:, t], in_=e_f[:, t])
                    nc.vector.scalar_tensor_tensor(
                        out=dumf[:],
                        in0=e_f[:, t],
                        scalar=1.0,
                        in1=e_f[:, t],
                        op0=ALU.mult,
                        op1=ALU.mult,
                        accum_out=sse[:, t : t + 1],
                    )
                for j in range(4):
                    ps = pst_p.tile([P, 4 * P], tp_dt, tag="pst")
                    for s in range(4):
                        t = 4 * q + s
                        nc.tensor.transpose(
                            ps[:, s * P : (s + 1) * P],
                            e_bf[:, t, j * P : (j + 1) * P],
                            ident[:],
                        )
                    nc.vector.tensor_copy(
                        out=e_T[:, j, q * 4 * P : (q + 1) * 4 * P], in_=ps[:]
                    )
                sl = slice(4 * q, 4 * (q + 1))
                nc.scalar.activation(lne[:, sl], sse[:, sl], AF.Ln)
                nc.scalar.activation(rse[:, sl], lne[:, sl], AF.Exp, scale=-0.5)
                nc.vector.tensor_scalar(
                    out=srse[:, sl], in0=rse[:, sl], scalar1=SCALE,
                    scalar2=None, op0=ALU.mult,
                )


            # ---- ground-truth path (spread across the group loop) ----
            gti = persist.tile([P, NT], i32)
            gtm = persist.tile([P, NT], f32)
            dot = persist.tile([P, NT], f32)
            ssg = persist.tile([P, NT], f32)

            def emit_gt_load():
                nc.sync.dma_start(gti[:], gti_d)
                nc.sync.dma_start(gtm[:], gtm_d)

            def emit_gt_chunk(tlist):
                for t in tlist:
                    wg = gat_p.tile([P, D], f32, tag="wg")
                    nc.gpsimd.indirect_dma_start(
                        out=wg[:],
                        out_offset=None,
                        in_=w_d,
                        in_offset=bass.IndirectOffsetOnAxis(
                            ap=gti[:, t : t + 1], axis=0
                        ),
                    )
                    nc.vector.scalar_tensor_tensor(
                        out=dumf[:],
                        in0=wg[:],
                        scalar=1.0,
                        in1=e_f[:, t],
                        op0=ALU.mult,
                        op1=ALU.mult,
                        accum_out=dot[:, t : t + 1],
                    )
                    nc.vector.scalar_tensor_tensor(
                        out=dumf[:],
                        in0=wg[:],
                        scalar=1.0,
                        in1=wg[:],
                        op0=ALU.mult,
                        op1=ALU.mult,
                        accum_out=ssg[:, t : t + 1],
                    )

            def emit_gt_tail():
                lng = persist.tile([P, NT], f32)
                rsg = persist.tile([P, NT], f32)
                nc.scalar.activation(lng[:], ssg[:], AF.Ln)
                nc.scalar.activation(rsg[:], lng[:], AF.Exp, scale=-0.5)
                gtc = persist.tile([P, NT], f32)
                nc.vector.tensor_tensor(
                    out=gtc[:], in0=dot[:], in1=rsg[:], op=ALU.mult
                )
                nc.vector.tensor_tensor(
                    out=gtc[:], in0=gtc[:], in1=rse[:], op=ALU.mult
                )
                nc.vector.tensor_tensor(
                    out=gtc[:], in0=gtc[:], in1=gtm[:], op=ALU.mult
                )
                nc.sync.dma_start(g_d, gtc[:])

            # ---- main streaming loop over class groups ----
            sexp = persist.tile([P, NT * NG], f32)
            for gi, (c0, n_sub, width, real) in enumerate(groups):
                if gi == 0:
                    wr = wr0
                else:
                    wr = wraw_p.tile([P, MAX_SUB, D], f32, tag="wr")
                    nc.sync.dma_start(
                        wr[:, :n_sub],
                        w_d[c0 : c0 + width].rearrange("(s p) d -> p s d", p=P),
                    )
                ssw = stat_p.tile([P, MAX_SUB], f32, tag="ssw")
                for s in range(n_sub):
                    nc.vector.scalar_tensor_tensor(
                        out=dumf[:],
                        in0=wr[:, s],
                        scalar=1.0,
                        in1=wr[:, s],
                        op0=ALU.mult,
                        op1=ALU.mult,
                        accum_out=ssw[:, s : s + 1],
                    )
                lnw = stat_p.tile([P, MAX_SUB], f32, tag="lnw")
                rsw = stat_p.tile([P, MAX_SUB], f32, tag="rsw")
                nc.scalar.activation(lnw[:, :n_sub], ssw[:, :n_sub], AF.Ln)
                nc.scalar.activation(
                    rsw[:, :n_sub], lnw[:, :n_sub], AF.Exp, scale=-0.5
                )
                wb = wbf_p.tile([P, MAX_SUB, D], tp_dt, tag="wb")
                for s in range(n_sub):
                    nc.vector.tensor_scalar(
                        out=wb[:, s],
                        in0=wr[:, s],
                        scalar1=rsw[:, s : s + 1],
                        scalar2=None,
                        op0=ALU.mult,
                    )
                # transpose to [d, c] layout (bf16), cast to fp8 in the copy
                wt = wt_p.tile([P, 4, GROUP_COLS], mm_dt, tag="wt")
                for j in range(4):
                    for qq in range((n_sub + 3) // 4):
                        ps = pst_p.tile([P, 4 * P], tp_dt, tag="pst")
                        hi = min(4, n_sub - qq * 4)
                        for s2 in range(hi):
                            s = qq * 4 + s2
                            nc.tensor.transpose(
                                ps[:, s2 * P : (s2 + 1) * P],
                                wb[:, s, j * P : (j + 1) * P],
                                ident[:],
                            )
                        nc.vector.tensor_copy(
                            out=wt[:, j, qq * 4 * P : qq * 4 * P + hi * P],
                            in_=ps[:, : hi * P],
                        )
                n_chunks = width // 512
                for t in range(NT):
                    pb = pb_p.tile([P, GROUP_COLS], f32, tag="pb")
                    if use_fp8:
                        for j in range(2):
                            for cc in range(n_chunks):
                                nc.tensor.matmul(
                                    pb[:, cc * 512 : (cc + 1) * 512],
                                    lhsT=e_T[
                                        :, 2 * j : 2 * j + 2, t * P : (t + 1) * P
                                    ],
                                    rhs=wt[
                                        :,
                                        2 * j : 2 * j + 2,
                                        cc * 512 : (cc + 1) * 512,
                                    ],
                                    start=(j == 0),
                                    stop=(j == 1),
                                    perf_mode=DR,
                                )
                    else:
                        for j in range(4):
                            for cc in range(n_chunks):
                                nc.tensor.matmul(
                                    pb[:, cc * 512 : (cc + 1) * 512],
                                    lhsT=e_T[:, j, t * P : (t + 1) * P],
                                    rhs=wt[:, j, cc * 512 : (cc + 1) * 512],
                                    start=(j == 0),
                                    stop=(j == 3),
                                )
                    du = dump_p.tile([P, GROUP_COLS], bf16, tag="du")
                    nc.scalar.activation(
                        du[:, :real],
                        pb[:, :real],
                        AF.Exp,
                        scale=srse[:, t : t + 1],
                        bias=negstab[:, :1],
                        accum_out=sexp[:, t * NG + gi : t * NG + gi + 1],
                    )
                if _BUILD_OPTS.get("gt", True):
                    if gi == 1:
                        emit_gt_load()
                    if gi in (2, 3, 4, 5):
                        emit_gt_chunk(range(4 * (gi - 2), 4 * (gi - 1)))
                    if gi == 6:
                        emit_gt_tail()

            spart = persist.tile([P, NT], f32)
            for t in range(NT):
                nc.vector.tensor_reduce(
                    spart[:, t : t + 1],
                    sexp[:, t * NG : (t + 1) * NG],
                    AX.X,
                    ALU.add,
                )
            nc.sync.dma_start(s_d, spart[:])

    nc.compile()
    _CACHE["nc"] = nc
    return nc


def run(embedding, ground_truth, weight, trace=False):
    """Run the sharded device kernel; returns (loss_scalar, BassKernelResults)."""
    import concourse.bass_utils as bass_utils

    if trace:
        _install_ntff_shim()

    nc = _build()

    emb = np.ascontiguousarray(np.asarray(embedding, dtype=np.float32))
    w_full = np.ascontiguousarray(np.asarray(weight, dtype=np.float32))
    gt = np.asarray(ground_truth).astype(np.int64)

    in_maps = []
    for k in range(N_CORES):
        lo = k * C_SHARD
        wshard = np.empty((C_PAD, D), dtype=np.float32)
        wshard[:C_SHARD] = w_full[lo : lo + C_SHARD]
        wshard[C_SHARD:] = 1.0  # pad rows; excluded from the exp reduction
        loc = gt - lo
        mask = (loc >= 0) & (loc < C_SHARD)
        idx = np.clip(loc, 0, C_SHARD - 1).astype(np.int32)
        in_maps.append(
            {
                "w": wshard,
                "emb": emb,
                "gt_idx": np.ascontiguousarray(idx.reshape(NT, P).T),
                "gt_mask": np.ascontiguousarray(
                    mask.reshape(NT, P).T.astype(np.float32)
                ),
            }
        )

    kwargs = {}
    if trace:
        import os

        os.environ["BASS_PERFETTO_PROFILE_ALL_CORES"] = "1"
        kwargs = dict(trace=True, trace_cores=list(range(N_CORES)), stitch_traces=False)

    res = bass_utils.run_bass_kernel_spmd(
        nc, in_maps, core_ids=list(range(N_CORES)), **kwargs
    )

    S = np.zeros(N, dtype=np.float64)
    cg = np.zeros(N, dtype=np.float64)
    for k in range(N_CORES):
        S += res.results[k]["s_out"].astype(np.float64).T.reshape(N)
        cg += res.results[k]["g_out"].astype(np.float64).T.reshape(N)

    lse = STAB + np.log(
        S - np.exp(SCALE * cg - STAB) + np.exp(SCALE * cg - SCALE * MARGIN - STAB)
    )
    nll = lse - (SCALE * cg - SCALE * MARGIN)
    loss = np.float32(nll.mean())
    return loss, res


def kernel(embedding, ground_truth, weight):
    loss, _ = run(embedding, ground_truth, weight, trace=False)
    return np.asarray(loss, dtype=np.float32)


# revision 12
# speedup vs baseline: 3.1632x; 1.1199x over previous
"""CosFace loss (N=2048, D=512, C=100000) on 8 Trainium2 NeuronCores.

Strategy (classifier/tensor parallel): shard the class dimension across the 8
cores (12500 classes each, padded to 12800). Each core streams its weight
shard once from HBM, computes cos = norm(emb) @ norm(w_shard).T in fp8e4
(DoubleRow, 2x PE rate) on the tensor engine, and reduces
sum_c exp(30*cos - 30) per batch row with the scalar engine's fused
exp+accumulate (fixed stabilizer 30 >= max logit since cos <= 1, so no max
pass). The embedding's l2-normalization is folded into the EXP's per-partition
scale (scale_n = 30 / ||e_n||), so the embedding path is just cast+transpose.
Weight rows are normalized on-device (sum-squares on DVE, rsqrt via ACT
Ln/Exp, multiply+cast on DVE), transposed d-major via PE in bf16, and cast to
fp8 during the PSUM->SBUF copy on DVE. Weight prep is software-pipelined two
groups ahead of the matmul+exp consumer loop.

The ground-truth logit for each row is computed exactly in fp32: the host
compacts the ~256 rows whose target class lives on this core into 384 padded
slots; the device indirect-DMA-gathers those weight rows AND embedding rows
from HBM (early, consumed late), then does 9 small fused dot/sum-square
reductions. The host scatters the per-core [128,3] results back to row order
and applies the CosFace margin + logsumexp formula in float64:

  lse_n = 30 + log(S_n - exp(30 c_n - 30) + exp(30 c_n - 12 - 30))
  nll_n = lse_n - (30 c_n - 12),  loss = mean_n nll_n

where S_n = sum_c exp(30 cos_nc - 30) (unmodified) and c_n = cos at the
target class. This is algebraically identical to softmax-CE with the margin
one-hot.
"""

import numpy as np

# Problem geometry (hardcoded per contract).
N, D, C = 2048, 512, 100000
P = 128
N_CORES = 8
C_SHARD = C // N_CORES  # 12500
C_PAD = 12800  # padded shard size: 100 tiles of 128
NT = N // P  # 16 batch tiles
SCALE = 30.0
MARGIN = 0.4
STAB = 30.0  # logsumexp stabilizer; valid since cos <= 1
GROUP_COLS = 1536  # classes per PSUM accumulation group (3 banks)
MAX_SUB = GROUP_COLS // P
GT_COLS = 3  # gathered ground-truth slots: 128*3 = 384 >= max owned rows

_CACHE = {}

# Debug knobs (bisecting hardware failures): set before first _build().
_BUILD_OPTS = {"gt": True, "ngroups": None, "fp8": True}


def _groups():
    # Processing order: the mostly-pad tail region (real=212) first -- its
    # small width makes the first matmul+EXP start early -- then the seven
    # full 1536 groups, then 1024/512 so the pipeline tail drains fast.
    plan = [(12288, 512), (0, 1536), (1536, 1536), (3072, 1536), (4608, 1536),
            (6144, 1536), (7680, 1536), (9216, 1536), (10752, 1024),
            (11776, 512)]
    gs = []
    for c0, w in plan:
        real = max(0, min(C_SHARD - c0, w))
        gs.append((c0, w // P, w, real))
    return gs


def _install_ntff_shim():
    """Register the axon NTFF profile hook if the image's antenv lacks it."""
    import sys
    import types

    try:
        from antenv.axon_hooks import get_axon_ntff_profile_hook  # noqa: F401

        return
    except ImportError:
        pass
    mod = types.ModuleType("antenv.axon_hooks")
    state = {"hook": None}
    mod.set_axon_ntff_profile_hook = lambda h: state.__setitem__("hook", h)
    mod.get_axon_ntff_profile_hook = lambda: state["hook"]
    sys.modules["antenv.axon_hooks"] = mod
    try:
        from trn_agent_boot.trn_boot import _ntff_profile_via_ctypes

        mod.set_axon_ntff_profile_hook(
            _ntff_profile_via_ctypes("/opt/axon/libaxon_pjrt.so")
        )
    except Exception:
        pass


def _build():
    if "nc" in _CACHE:
        return _CACHE["nc"]

    import concourse.bass as bass
    import concourse.tile as tile
    from concourse import bacc, mybir
    from concourse.masks import make_identity

    # Restrict the activation-table universe to the one set that contains
    # every function we use (Ln, Exp) so the compiler emits a single
    # ACT_TABLE_LOAD instead of thrashing between sets (~2.7us per switch).
    import concourse.hw_specs as hw_specs

    if not getattr(bacc, "_cosface_act_patch", False):
        _orig_get_tables = hw_specs.get_activation_tables

        def _one_set(arch):
            t = _orig_get_tables(arch)
            keep = {"Exp", "Ln", "Square"}
            return {
                name: (
                    funcs
                    if name == "natural_log_exp_and_others"
                    else {f for f in funcs if f.name not in keep}
                )
                for name, funcs in t.items()
            }

        bacc.get_activation_tables = _one_set
        bacc._cosface_act_patch = True

    f32 = mybir.dt.float32
    bf16 = mybir.dt.bfloat16
    i32 = mybir.dt.int32
    AF = mybir.ActivationFunctionType
    ALU = mybir.AluOpType
    AX = mybir.AxisListType
    use_fp8 = _BUILD_OPTS.get("fp8", False)
    mm_dt = mybir.dt.float8e4 if use_fp8 else bf16
    DR = mybir.MatmulPerfMode.DoubleRow

    groups = _groups()
    if _BUILD_OPTS.get("ngroups") is not None:
        groups = groups[: _BUILD_OPTS["ngroups"]]
    NG = len(groups)
    use_gt = _BUILD_OPTS.get("gt", True)

    nc = bacc.Bacc(
        "TRN2", target_bir_lowering=False, debug=False, num_devices=N_CORES
    )
    w_d = nc.dram_tensor("w", [C_PAD, D], f32, kind="ExternalInput").ap()
    emb_d = nc.dram_tensor("emb", [N, D], f32, kind="ExternalInput").ap()
    gn_d = nc.dram_tensor("gn_idx", [P, GT_COLS], i32, kind="ExternalInput").ap()
    gc_d = nc.dram_tensor("gc_idx", [P, GT_COLS], i32, kind="ExternalInput").ap()
    s_d = nc.dram_tensor("s_out", [P, NT], f32, kind="ExternalOutput").ap()
    g_d = nc.dram_tensor("g_out", [P, GT_COLS], f32, kind="ExternalOutput").ap()

    with tile.TileContext(nc) as tc:
        with (
            tc.tile_pool(name="persist", bufs=1) as persist,
            tc.tile_pool(name="wraw", bufs=3) as wraw_p,
            tc.tile_pool(name="wbf", bufs=2) as wbf_p,
            tc.tile_pool(name="wt", bufs=3) as wt_p,
            tc.tile_pool(name="stat", bufs=3) as stat_p,
            tc.tile_pool(name="dump", bufs=2) as dump_p,
            tc.tile_pool(name="pst", bufs=2, space="PSUM") as pst_p,
            tc.tile_pool(name="pbp", bufs=2, space="PSUM") as pb_p,
        ):
            # Transposes run in bf16 (fp8 PE transpose needs element-step-2
            # output); the psum->sbuf copy casts to mm_dt for the matmuls.
            tp_dt = bf16
            ident = persist.tile([P, P], tp_dt)
            make_identity(nc, ident[:])
            negstab = persist.tile([P, 1], f32)
            nc.vector.memset(negstab[:], -STAB)
            dumf = persist.tile([P, D], f32)  # DVE accum dummy

            # ---- weight group prep, software-pipelined ----
            def emit_wdma(gi):
                c0, n_sub, width, _ = groups[gi]
                wr = wraw_p.tile([P, MAX_SUB, D], f32, tag="wr")
                nc.sync.dma_start(
                    wr[:, :n_sub],
                    w_d[c0 : c0 + width].rearrange("(s p) d -> p s d", p=P),
                )
                return wr

            def emit_wprep(gi, wr):
                c0, n_sub, width, _ = groups[gi]
                ssw = stat_p.tile([P, MAX_SUB], f32, tag="ssw")
                for s in range(n_sub):
                    nc.vector.scalar_tensor_tensor(
                        out=dumf[:],
                        in0=wr[:, s],
                        scalar=1.0,
                        in1=wr[:, s],
                        op0=ALU.mult,
                        op1=ALU.mult,
                        accum_out=ssw[:, s : s + 1],
                    )
                lnw = stat_p.tile([P, MAX_SUB], f32, tag="lnw")
                rsw = stat_p.tile([P, MAX_SUB], f32, tag="rsw")
                nc.scalar.activation(lnw[:, :n_sub], ssw[:, :n_sub], AF.Ln)
                nc.scalar.activation(
                    rsw[:, :n_sub], lnw[:, :n_sub], AF.Exp, scale=-0.5
                )
                wb = wbf_p.tile([P, MAX_SUB, D], tp_dt, tag="wb")
                for s in range(n_sub):
                    nc.vector.tensor_scalar(
                        out=wb[:, s],
                        in0=wr[:, s],
                        scalar1=rsw[:, s : s + 1],
                        scalar2=None,
                        op0=ALU.mult,
                    )
                wt = wt_p.tile([P, 4, GROUP_COLS], mm_dt, tag="wt")
                for j in range(4):
                    for qq in range((n_sub + 3) // 4):
                        ps = pst_p.tile([P, 4 * P], tp_dt, tag="pst")
                        hi = min(4, n_sub - qq * 4)
                        for s2 in range(hi):
                            s = qq * 4 + s2
                            nc.tensor.transpose(
                                ps[:, s2 * P : (s2 + 1) * P],
                                wb[:, s, j * P : (j + 1) * P],
                                ident[:],
                            )
                        nc.vector.tensor_copy(
                            out=wt[:, j, qq * 4 * P : qq * 4 * P + hi * P],
                            in_=ps[:, : hi * P],
                        )
                return wt

            wr_pend = {0: emit_wdma(0), 1: emit_wdma(1)}
            wt_ready = {}

            # ---- embedding: chunked load, cast, transpose ----
            # l2-normalization of e is folded into the EXP scale (srse).
            e_f = persist.tile([P, NT, D], f32)
            e_bf = persist.tile([P, NT, D], tp_dt)
            sse = persist.tile([P, NT], f32)
            e_T = persist.tile([P, 4, N], mm_dt)
            lne = persist.tile([P, NT], f32)
            rse = persist.tile([P, NT], f32)
            srse = persist.tile([P, NT], f32)
            emb_r = emb_d.rearrange("(t p) d -> p t d", p=P)
            for q in range(4):
                nc.sync.dma_start(
                    e_f[:, 4 * q : 4 * (q + 1)], emb_r[:, 4 * q : 4 * (q + 1)]
                )
                for s in range(4):
                    t = 4 * q + s
                    nc.vector.tensor_copy(out=e_bf[:, t], in_=e_f[:, t])
                    nc.vector.scalar_tensor_tensor(
                        out=dumf[:],
                        in0=e_f[:, t],
                        scalar=1.0,
                        in1=e_f[:, t],
                        op0=ALU.mult,
                        op1=ALU.mult,
                        accum_out=sse[:, t : t + 1],
                    )
                for j in range(4):
                    ps = pst_p.tile([P, 4 * P], tp_dt, tag="pst")
                    for s in range(4):
                        t = 4 * q + s
                        nc.tensor.transpose(
                            ps[:, s * P : (s + 1) * P],
                            e_bf[:, t, j * P : (j + 1) * P],
                            ident[:],
                        )
                    nc.vector.tensor_copy(
                        out=e_T[:, j, q * 4 * P : (q + 1) * 4 * P], in_=ps[:]
                    )
                sl = slice(4 * q, 4 * (q + 1))
                nc.scalar.activation(lne[:, sl], sse[:, sl], AF.Ln)
                nc.scalar.activation(rse[:, sl], lne[:, sl], AF.Exp, scale=-0.5)
                nc.vector.tensor_scalar(
                    out=srse[:, sl], in0=rse[:, sl], scalar1=SCALE,
                    scalar2=None, op0=ALU.mult,
                )
                if q == 0:
                    wt_ready[0] = emit_wprep(0, wr_pend.pop(0))
                if q == 1:
                    wr_pend[2] = emit_wdma(2)
                if q == 2:
                    wt_ready[1] = emit_wprep(1, wr_pend.pop(1))

            # ---- ground-truth gathers (issued early, consumed late) ----
            gnt = persist.tile([P, GT_COLS], i32)
            gct = persist.tile([P, GT_COLS], i32)
            gw = persist.tile([P, GT_COLS, D], f32)
            ge = persist.tile([P, GT_COLS, D], f32)

            def emit_gt_gather():
                nc.sync.dma_start(gnt[:], gn_d)
                nc.sync.dma_start(gct[:], gc_d)
                for col in range(GT_COLS):
                    nc.gpsimd.indirect_dma_start(
                        out=gw[:, col],
                        out_offset=None,
                        in_=w_d,
                        in_offset=bass.IndirectOffsetOnAxis(
                            ap=gct[:, col : col + 1], axis=0
                        ),
                    )
                    nc.gpsimd.indirect_dma_start(
                        out=ge[:, col],
                        out_offset=None,
                        in_=emb_d,
                        in_offset=bass.IndirectOffsetOnAxis(
                            ap=gnt[:, col : col + 1], axis=0
                        ),
                    )

            def emit_gt_compute():
                gdot = persist.tile([P, GT_COLS], f32)
                gssw = persist.tile([P, GT_COLS], f32)
                gsse = persist.tile([P, GT_COLS], f32)
                for col in range(GT_COLS):
                    nc.vector.scalar_tensor_tensor(
                        out=dumf[:], in0=ge[:, col], scalar=1.0, in1=gw[:, col],
                        op0=ALU.mult, op1=ALU.mult,
                        accum_out=gdot[:, col : col + 1],
                    )
                    nc.vector.scalar_tensor_tensor(
                        out=dumf[:], in0=gw[:, col], scalar=1.0, in1=gw[:, col],
                        op0=ALU.mult, op1=ALU.mult,
                        accum_out=gssw[:, col : col + 1],
                    )
                    nc.vector.scalar_tensor_tensor(
                        out=dumf[:], in0=ge[:, col], scalar=1.0, in1=ge[:, col],
                        op0=ALU.mult, op1=ALU.mult,
                        accum_out=gsse[:, col : col + 1],
                    )
                lgw = persist.tile([P, GT_COLS], f32)
                rgw = persist.tile([P, GT_COLS], f32)
                lge = persist.tile([P, GT_COLS], f32)
                rge = persist.tile([P, GT_COLS], f32)
                nc.scalar.activation(lgw[:], gssw[:], AF.Ln)
                nc.scalar.activation(rgw[:], lgw[:], AF.Exp, scale=-0.5)
                nc.scalar.activation(lge[:], gsse[:], AF.Ln)
                nc.scalar.activation(rge[:], lge[:], AF.Exp, scale=-0.5)
                gtc = persist.tile([P, GT_COLS], f32)
                nc.vector.tensor_tensor(
                    out=gtc[:], in0=gdot[:], in1=rgw[:], op=ALU.mult
                )
                nc.vector.tensor_tensor(
                    out=gtc[:], in0=gtc[:], in1=rge[:], op=ALU.mult
                )
                nc.sync.dma_start(g_d, gtc[:])

            # ---- main streaming loop over class groups ----
            sexp = persist.tile([P, NT * NG], f32)
            spart = persist.tile([P, NT], f32)
            for gi, (c0, n_sub, width, real) in enumerate(groups):
                wt = wt_ready.pop(gi)
                n_chunks = width // 512
                for t in range(NT):
                    pb = pb_p.tile([P, GROUP_COLS], f32, tag="pb")
                    if use_fp8:
                        for j in range(2):
                            for cc in range(n_chunks):
                                nc.tensor.matmul(
                                    pb[:, cc * 512 : (cc + 1) * 512],
                                    lhsT=e_T[
                                        :, 2 * j : 2 * j + 2, t * P : (t + 1) * P
                                    ],
                                    rhs=wt[
                                        :,
                                        2 * j : 2 * j + 2,
                                        cc * 512 : (cc + 1) * 512,
                                    ],
                                    start=(j == 0),
                                    stop=(j == 1),
                                    perf_mode=DR,
                                )
                    else:
                        for j in range(4):
                            for cc in range(n_chunks):
                                nc.tensor.matmul(
                                    pb[:, cc * 512 : (cc + 1) * 512],
                                    lhsT=e_T[:, j, t * P : (t + 1) * P],
                                    rhs=wt[:, j, cc * 512 : (cc + 1) * 512],
                                    start=(j == 0),
                                    stop=(j == 3),
                                )
                    du = dump_p.tile([P, GROUP_COLS], bf16, tag="du")
                    nc.scalar.activation(
                        du[:, :real],
                        pb[:, :real],
                        AF.Exp,
                        scale=srse[:, t : t + 1],
                        bias=negstab[:, :1],
                        accum_out=sexp[:, t * NG + gi : t * NG + gi + 1],
                    )
                    if gi == NG - 1:
                        nc.vector.tensor_reduce(
                            spart[:, t : t + 1],
                            sexp[:, t * NG : (t + 1) * NG],
                            AX.X,
                            ALU.add,
                        )
                # pipeline: DMA 3 ahead, prep 2 ahead
                if gi + 3 < NG:
                    wr_pend[gi + 3] = emit_wdma(gi + 3)
                if gi + 2 < NG:
                    wt_ready[gi + 2] = emit_wprep(gi + 2, wr_pend.pop(gi + 2))
                if use_gt:
                    if gi == 0:
                        emit_gt_gather()
                    if gi == 7:
                        emit_gt_compute()

            nc.sync.dma_start(s_d, spart[:])

    nc.compile()
    _CACHE["nc"] = nc
    return nc


def run(embedding, ground_truth, weight, trace=False):
    """Run the sharded device kernel; returns (loss_scalar, BassKernelResults)."""
    import concourse.bass_utils as bass_utils

    if trace:
        _install_ntff_shim()

    nc = _build()

    emb = np.ascontiguousarray(np.asarray(embedding, dtype=np.float32))
    w_full = np.ascontiguousarray(np.asarray(weight, dtype=np.float32))
    gt = np.asarray(ground_truth).astype(np.int64)

    K = P * GT_COLS
    in_maps = []
    owned_lists = []
    for k in range(N_CORES):
        lo = k * C_SHARD
        wshard = np.empty((C_PAD, D), dtype=np.float32)
        wshard[:C_SHARD] = w_full[lo : lo + C_SHARD]
        wshard[C_SHARD:] = 1.0  # pad rows; excluded from the exp reduction
        loc = gt - lo
        mask = (loc >= 0) & (loc < C_SHARD)
        owned = np.where(mask)[0]
        assert len(owned) <= K, f"core {k} owns {len(owned)} > {K} rows"
        owned_lists.append(owned)
        L = np.zeros(K, dtype=np.int64)
        L[: len(owned)] = owned
        gn = L.astype(np.int32)
        gc = np.clip(gt[L] - lo, 0, C_SHARD - 1).astype(np.int32)
        in_maps.append(
            {
                "w": wshard,
                "emb": emb,
                "gn_idx": np.ascontiguousarray(gn.reshape(GT_COLS, P).T),
                "gc_idx": np.ascontiguousarray(gc.reshape(GT_COLS, P).T),
            }
        )

    kwargs = {}
    if trace:
        import os

        os.environ["BASS_PERFETTO_PROFILE_ALL_CORES"] = "1"
        kwargs = dict(trace=True, trace_cores=list(range(N_CORES)), stitch_traces=False)

    res = bass_utils.run_bass_kernel_spmd(
        nc, in_maps, core_ids=list(range(N_CORES)), **kwargs
    )

    S = np.zeros(N, dtype=np.float64)
    cg = np.zeros(N, dtype=np.float64)
    for k in range(N_CORES):
        S += res.results[k]["s_out"].astype(np.float64).T.reshape(N)
        gvals = res.results[k]["g_out"].astype(np.float64).T.reshape(K)
        owned = owned_lists[k]
        cg[owned] = gvals[: len(owned)]

    lse = STAB + np.log(
        S - np.exp(SCALE * cg - STAB) + np.exp(SCALE * cg - SCALE * MARGIN - STAB)
    )
    nll = lse - (SCALE * cg - SCALE * MARGIN)
    loss = np.float32(nll.mean())
    return loss, res


def kernel(embedding, ground_truth, weight):
    loss, _ = run(embedding, ground_truth, weight, trace=False)
    return np.asarray(loss, dtype=np.float32)


# revision 13
# speedup vs baseline: 3.2506x; 1.0276x over previous
"""CosFace loss (N=2048, D=512, C=100000) on 8 Trainium2 NeuronCores.

Strategy (classifier/tensor parallel): shard the class dimension across the 8
cores (12500 classes each, padded to 12800). Each core streams its weight
shard once from HBM, computes cos = norm(emb) @ norm(w_shard).T in fp8e4
(DoubleRow, 2x PE rate) on the tensor engine, and reduces
sum_c exp(30*cos - 30) per batch row with the scalar engine's fused
exp+accumulate (fixed stabilizer 30 >= max logit since cos <= 1, so no max
pass). The embedding's l2-normalization is folded into the EXP's per-partition
scale (scale_n = 30 / ||e_n||), so the embedding path is just cast+transpose.
Weight rows are normalized on-device (sum-squares on DVE, rsqrt via ACT
Ln/Exp, multiply+cast on DVE), transposed d-major via PE in bf16, and cast to
fp8 during the PSUM->SBUF copy on DVE. Weight prep is software-pipelined two
groups ahead of the matmul+exp consumer loop.

The ground-truth logit for each row is computed exactly in fp32: the host
compacts the ~256 rows whose target class lives on this core into 384 padded
slots; the device indirect-DMA-gathers those weight rows AND embedding rows
from HBM (early, consumed late), then does 9 small fused dot/sum-square
reductions. The host scatters the per-core [128,3] results back to row order
and applies the CosFace margin + logsumexp formula in float64:

  lse_n = 30 + log(S_n - exp(30 c_n - 30) + exp(30 c_n - 12 - 30))
  nll_n = lse_n - (30 c_n - 12),  loss = mean_n nll_n

where S_n = sum_c exp(30 cos_nc - 30) (unmodified) and c_n = cos at the
target class. This is algebraically identical to softmax-CE with the margin
one-hot.
"""

import numpy as np

# Problem geometry (hardcoded per contract).
N, D, C = 2048, 512, 100000
P = 128
N_CORES = 8
C_SHARD = C // N_CORES  # 12500
C_PAD = 12800  # padded shard size: 100 tiles of 128
NT = N // P  # 16 batch tiles
SCALE = 30.0
MARGIN = 0.4
STAB = 30.0  # logsumexp stabilizer; valid since cos <= 1
GROUP_COLS = 1536  # classes per PSUM accumulation group (3 banks)
MAX_SUB = GROUP_COLS // P
GT_COLS = 3  # gathered ground-truth slots: 128*3 = 384 >= max owned rows

_CACHE = {}

# Debug knobs (bisecting hardware failures): set before first _build().
_BUILD_OPTS = {"gt": True, "ngroups": None, "fp8": True}


def _groups():
    # Processing order: the mostly-pad tail region (real=212) first -- its
    # small width makes the first matmul+EXP start early -- then the seven
    # full 1536 groups, then 1024/512 so the pipeline tail drains fast.
    plan = [(12288, 512), (0, 1536), (1536, 1536), (3072, 1536), (4608, 1536),
            (6144, 1536), (7680, 1536), (9216, 1536), (10752, 1024),
            (11776, 512)]
    gs = []
    for c0, w in plan:
        real = max(0, min(C_SHARD - c0, w))
        gs.append((c0, w // P, w, real))
    return gs


def _install_ntff_shim():
    """Register the axon NTFF profile hook if the image's antenv lacks it."""
    import sys
    import types

    try:
        from antenv.axon_hooks import get_axon_ntff_profile_hook  # noqa: F401

        return
    except ImportError:
        pass
    mod = types.ModuleType("antenv.axon_hooks")
    state = {"hook": None}
    mod.set_axon_ntff_profile_hook = lambda h: state.__setitem__("hook", h)
    mod.get_axon_ntff_profile_hook = lambda: state["hook"]
    sys.modules["antenv.axon_hooks"] = mod
    try:
        from trn_agent_boot.trn_boot import _ntff_profile_via_ctypes

        mod.set_axon_ntff_profile_hook(
            _ntff_profile_via_ctypes("/opt/axon/libaxon_pjrt.so")
        )
    except Exception:
        pass


def _build():
    if "nc" in _CACHE:
        return _CACHE["nc"]

    import concourse.bass as bass
    import concourse.tile as tile
    from concourse import bacc, mybir
    from concourse.masks import make_identity

    # Restrict the activation-table universe to the one set that contains
    # every function we use (Ln, Exp) so the compiler emits a single
    # ACT_TABLE_LOAD instead of thrashing between sets (~2.7us per switch).
    import concourse.hw_specs as hw_specs

    if not getattr(bacc, "_cosface_act_patch", False):
        _orig_get_tables = hw_specs.get_activation_tables

        def _one_set(arch):
            t = _orig_get_tables(arch)
            keep = {"Exp", "Ln", "Square"}
            return {
                name: (
                    funcs
                    if name == "natural_log_exp_and_others"
                    else {f for f in funcs if f.name not in keep}
                )
                for name, funcs in t.items()
            }

        bacc.get_activation_tables = _one_set
        bacc._cosface_act_patch = True

    f32 = mybir.dt.float32
    bf16 = mybir.dt.bfloat16
    i32 = mybir.dt.int32
    AF = mybir.ActivationFunctionType
    ALU = mybir.AluOpType
    AX = mybir.AxisListType
    use_fp8 = _BUILD_OPTS.get("fp8", False)
    mm_dt = mybir.dt.float8e4 if use_fp8 else bf16
    DR = mybir.MatmulPerfMode.DoubleRow

    groups = _groups()
    if _BUILD_OPTS.get("ngroups") is not None:
        groups = groups[: _BUILD_OPTS["ngroups"]]
    NG = len(groups)
    use_gt = _BUILD_OPTS.get("gt", True)

    nc = bacc.Bacc(
        "TRN2", target_bir_lowering=False, debug=False, num_devices=N_CORES
    )
    w_d = nc.dram_tensor("w", [C_PAD, D], f32, kind="ExternalInput").ap()
    emb_d = nc.dram_tensor("emb", [N, D], f32, kind="ExternalInput").ap()
    gn_d = nc.dram_tensor("gn_idx", [P, GT_COLS], i32, kind="ExternalInput").ap()
    gc_d = nc.dram_tensor("gc_idx", [P, GT_COLS], i32, kind="ExternalInput").ap()
    s_d = nc.dram_tensor("s_out", [P, NT], f32, kind="ExternalOutput").ap()
    g_d = nc.dram_tensor("g_out", [P, GT_COLS], f32, kind="ExternalOutput").ap()

    with tile.TileContext(nc) as tc:
        with (
            tc.tile_pool(name="persist", bufs=1) as persist,
            tc.tile_pool(name="wraw", bufs=3) as wraw_p,
            tc.tile_pool(name="wbf", bufs=2) as wbf_p,
            tc.tile_pool(name="wt", bufs=3) as wt_p,
            tc.tile_pool(name="stat", bufs=3) as stat_p,
            tc.tile_pool(name="dump", bufs=2) as dump_p,
            tc.tile_pool(name="pst", bufs=2, space="PSUM") as pst_p,
            tc.tile_pool(name="pbp", bufs=2, space="PSUM") as pb_p,
        ):
            # Transposes run in bf16 (fp8 PE transpose needs element-step-2
            # output); the psum->sbuf copy casts to mm_dt for the matmuls.
            tp_dt = bf16
            ident = persist.tile([P, P], tp_dt)
            make_identity(nc, ident[:])
            negstab = persist.tile([P, 1], f32)
            nc.vector.memset(negstab[:], -STAB)
            dumf = persist.tile([P, D], f32)  # DVE accum dummy

            # ---- weight group prep, software-pipelined ----
            def emit_wdma(gi):
                c0, n_sub, width, _ = groups[gi]
                wr = wraw_p.tile([P, MAX_SUB, D], f32, tag="wr")
                nc.sync.dma_start(
                    wr[:, :n_sub],
                    w_d[c0 : c0 + width].rearrange("(s p) d -> p s d", p=P),
                )
                return wr

            def emit_wprep(gi, wr):
                c0, n_sub, width, _ = groups[gi]
                ssw = stat_p.tile([P, MAX_SUB], f32, tag="ssw")
                for s in range(n_sub):
                    nc.vector.scalar_tensor_tensor(
                        out=dumf[:],
                        in0=wr[:, s],
                        scalar=1.0,
                        in1=wr[:, s],
                        op0=ALU.mult,
                        op1=ALU.mult,
                        accum_out=ssw[:, s : s + 1],
                    )
                lnw = stat_p.tile([P, MAX_SUB], f32, tag="lnw")
                rsw = stat_p.tile([P, MAX_SUB], f32, tag="rsw")
                nc.scalar.activation(lnw[:, :n_sub], ssw[:, :n_sub], AF.Ln)
                nc.scalar.activation(
                    rsw[:, :n_sub], lnw[:, :n_sub], AF.Exp, scale=-0.5
                )
                wb = wbf_p.tile([P, MAX_SUB, D], tp_dt, tag="wb")
                for s in range(n_sub):
                    nc.vector.tensor_scalar(
                        out=wb[:, s],
                        in0=wr[:, s],
                        scalar1=rsw[:, s : s + 1],
                        scalar2=None,
                        op0=ALU.mult,
                    )
                wt = wt_p.tile([P, 4, GROUP_COLS], mm_dt, tag="wt")
                for j in range(4):
                    for qq in range((n_sub + 3) // 4):
                        ps = pst_p.tile([P, 4 * P], tp_dt, tag="pst")
                        hi = min(4, n_sub - qq * 4)
                        for s2 in range(hi):
                            s = qq * 4 + s2
                            nc.tensor.transpose(
                                ps[:, s2 * P : (s2 + 1) * P],
                                wb[:, s, j * P : (j + 1) * P],
                                ident[:],
                            )
                        nc.vector.tensor_copy(
                            out=wt[:, j, qq * 4 * P : qq * 4 * P + hi * P],
                            in_=ps[:, : hi * P],
                        )
                return wt

            wr_pend = {}
            wt_ready = {}

            # ---- embedding: chunked load, cast, transpose ----
            # l2-normalization of e is folded into the EXP scale (srse).
            e_f = persist.tile([P, NT, D], f32)
            e_bf = persist.tile([P, NT, D], tp_dt)
            sse = persist.tile([P, NT], f32)
            e_T = persist.tile([P, 4, N], mm_dt)
            lne = persist.tile([P, NT], f32)
            rse = persist.tile([P, NT], f32)
            srse = persist.tile([P, NT], f32)
            emb_r = emb_d.rearrange("(t p) d -> p t d", p=P)
            # all four 1MB embedding chunks dispatch first, then the weight
            # loads (the e path feeds e_T which gates every matmul tile).
            for q in range(4):
                nc.sync.dma_start(
                    e_f[:, 4 * q : 4 * (q + 1)], emb_r[:, 4 * q : 4 * (q + 1)]
                )
            wr_pend[0] = emit_wdma(0)
            for q in range(4):
                for s in range(4):
                    t = 4 * q + s
                    nc.vector.tensor_copy(out=e_bf[:, t], in_=e_f[:, t])
                    nc.vector.scalar_tensor_tensor(
                        out=dumf[:],
                        in0=e_f[:, t],
                        scalar=1.0,
                        in1=e_f[:, t],
                        op0=ALU.mult,
                        op1=ALU.mult,
                        accum_out=sse[:, t : t + 1],
                    )
                for j in range(4):
                    ps = pst_p.tile([P, 4 * P], tp_dt, tag="pst")
                    for s in range(4):
                        t = 4 * q + s
                        nc.tensor.transpose(
                            ps[:, s * P : (s + 1) * P],
                            e_bf[:, t, j * P : (j + 1) * P],
                            ident[:],
                        )
                    nc.vector.tensor_copy(
                        out=e_T[:, j, q * 4 * P : (q + 1) * 4 * P], in_=ps[:]
                    )
                sl = slice(4 * q, 4 * (q + 1))
                nc.scalar.activation(lne[:, sl], sse[:, sl], AF.Ln)
                nc.scalar.activation(rse[:, sl], lne[:, sl], AF.Exp, scale=-0.5)
                nc.vector.tensor_scalar(
                    out=srse[:, sl], in0=rse[:, sl], scalar1=SCALE,
                    scalar2=None, op0=ALU.mult,
                )
                if q == 0:
                    wt_ready[0] = emit_wprep(0, wr_pend.pop(0))
                    wr_pend[1] = emit_wdma(1)
                if q == 1:
                    wr_pend[2] = emit_wdma(2)
                if q == 2:
                    wt_ready[1] = emit_wprep(1, wr_pend.pop(1))

            # ---- ground-truth gathers (issued early, consumed late) ----
            gnt = persist.tile([P, GT_COLS], i32)
            gct = persist.tile([P, GT_COLS], i32)
            gw = persist.tile([P, GT_COLS, D], f32)
            ge = persist.tile([P, GT_COLS, D], f32)

            def emit_gt_gather():
                nc.sync.dma_start(gnt[:], gn_d)
                nc.sync.dma_start(gct[:], gc_d)
                for col in range(GT_COLS):
                    nc.gpsimd.indirect_dma_start(
                        out=gw[:, col],
                        out_offset=None,
                        in_=w_d,
                        in_offset=bass.IndirectOffsetOnAxis(
                            ap=gct[:, col : col + 1], axis=0
                        ),
                    )
                    nc.gpsimd.indirect_dma_start(
                        out=ge[:, col],
                        out_offset=None,
                        in_=emb_d,
                        in_offset=bass.IndirectOffsetOnAxis(
                            ap=gnt[:, col : col + 1], axis=0
                        ),
                    )

            def emit_gt_compute():
                gdot = persist.tile([P, GT_COLS], f32)
                gssw = persist.tile([P, GT_COLS], f32)
                gsse = persist.tile([P, GT_COLS], f32)
                for col in range(GT_COLS):
                    nc.vector.scalar_tensor_tensor(
                        out=dumf[:], in0=ge[:, col], scalar=1.0, in1=gw[:, col],
                        op0=ALU.mult, op1=ALU.mult,
                        accum_out=gdot[:, col : col + 1],
                    )
                    nc.vector.scalar_tensor_tensor(
                        out=dumf[:], in0=gw[:, col], scalar=1.0, in1=gw[:, col],
                        op0=ALU.mult, op1=ALU.mult,
                        accum_out=gssw[:, col : col + 1],
                    )
                    nc.vector.scalar_tensor_tensor(
                        out=dumf[:], in0=ge[:, col], scalar=1.0, in1=ge[:, col],
                        op0=ALU.mult, op1=ALU.mult,
                        accum_out=gsse[:, col : col + 1],
                    )
                lgw = persist.tile([P, GT_COLS], f32)
                rgw = persist.tile([P, GT_COLS], f32)
                lge = persist.tile([P, GT_COLS], f32)
                rge = persist.tile([P, GT_COLS], f32)
                nc.scalar.activation(lgw[:], gssw[:], AF.Ln)
                nc.scalar.activation(rgw[:], lgw[:], AF.Exp, scale=-0.5)
                nc.scalar.activation(lge[:], gsse[:], AF.Ln)
                nc.scalar.activation(rge[:], lge[:], AF.Exp, scale=-0.5)
                gtc = persist.tile([P, GT_COLS], f32)
                nc.vector.tensor_tensor(
                    out=gtc[:], in0=gdot[:], in1=rgw[:], op=ALU.mult
                )
                nc.vector.tensor_tensor(
                    out=gtc[:], in0=gtc[:], in1=rge[:], op=ALU.mult
                )
                nc.sync.dma_start(g_d, gtc[:])

            # ---- main streaming loop over class groups ----
            sexp = persist.tile([P, NT * NG], f32)
            spart = persist.tile([P, NT], f32)
            for gi, (c0, n_sub, width, real) in enumerate(groups):
                wt = wt_ready.pop(gi)
                n_chunks = width // 512
                for t in range(NT):
                    pb = pb_p.tile([P, GROUP_COLS], f32, tag="pb")
                    if use_fp8:
                        for j in range(2):
                            for cc in range(n_chunks):
                                nc.tensor.matmul(
                                    pb[:, cc * 512 : (cc + 1) * 512],
                                    lhsT=e_T[
                                        :, 2 * j : 2 * j + 2, t * P : (t + 1) * P
                                    ],
                                    rhs=wt[
                                        :,
                                        2 * j : 2 * j + 2,
                                        cc * 512 : (cc + 1) * 512,
                                    ],
                                    start=(j == 0),
                                    stop=(j == 1),
                                    perf_mode=DR,
                                )
                    else:
                        for j in range(4):
                            for cc in range(n_chunks):
                                nc.tensor.matmul(
                                    pb[:, cc * 512 : (cc + 1) * 512],
                                    lhsT=e_T[:, j, t * P : (t + 1) * P],
                                    rhs=wt[:, j, cc * 512 : (cc + 1) * 512],
                                    start=(j == 0),
                                    stop=(j == 3),
                                )
                    du = dump_p.tile([P, GROUP_COLS], bf16, tag="du")
                    nc.scalar.activation(
                        du[:, :real],
                        pb[:, :real],
                        AF.Exp,
                        scale=srse[:, t : t + 1],
                        bias=negstab[:, :1],
                        accum_out=sexp[:, t * NG + gi : t * NG + gi + 1],
                    )
                    if gi == NG - 1:
                        nc.vector.tensor_reduce(
                            spart[:, t : t + 1],
                            sexp[:, t * NG : (t + 1) * NG],
                            AX.X,
                            ALU.add,
                        )
                # pipeline: DMA 3 ahead, prep 2 ahead
                if gi + 3 < NG:
                    wr_pend[gi + 3] = emit_wdma(gi + 3)
                if gi + 2 < NG:
                    wt_ready[gi + 2] = emit_wprep(gi + 2, wr_pend.pop(gi + 2))
                if use_gt:
                    if gi == 0:
                        emit_gt_gather()
                    if gi == 7:
                        emit_gt_compute()

            nc.sync.dma_start(s_d, spart[:])

    nc.compile()
    _CACHE["nc"] = nc
    return nc


def run(embedding, ground_truth, weight, trace=False):
    """Run the sharded device kernel; returns (loss_scalar, BassKernelResults)."""
    import concourse.bass_utils as bass_utils

    if trace:
        _install_ntff_shim()

    nc = _build()

    emb = np.ascontiguousarray(np.asarray(embedding, dtype=np.float32))
    w_full = np.ascontiguousarray(np.asarray(weight, dtype=np.float32))
    gt = np.asarray(ground_truth).astype(np.int64)

    K = P * GT_COLS
    in_maps = []
    owned_lists = []
    for k in range(N_CORES):
        lo = k * C_SHARD
        wshard = np.empty((C_PAD, D), dtype=np.float32)
        wshard[:C_SHARD] = w_full[lo : lo + C_SHARD]
        wshard[C_SHARD:] = 1.0  # pad rows; excluded from the exp reduction
        loc = gt - lo
        mask = (loc >= 0) & (loc < C_SHARD)
        owned = np.where(mask)[0]
        assert len(owned) <= K, f"core {k} owns {len(owned)} > {K} rows"
        owned_lists.append(owned)
        L = np.zeros(K, dtype=np.int64)
        L[: len(owned)] = owned
        gn = L.astype(np.int32)
        gc = np.clip(gt[L] - lo, 0, C_SHARD - 1).astype(np.int32)
        in_maps.append(
            {
                "w": wshard,
                "emb": emb,
                "gn_idx": np.ascontiguousarray(gn.reshape(GT_COLS, P).T),
                "gc_idx": np.ascontiguousarray(gc.reshape(GT_COLS, P).T),
            }
        )

    kwargs = {}
    if trace:
        import os

        os.environ["BASS_PERFETTO_PROFILE_ALL_CORES"] = "1"
        kwargs = dict(trace=True, trace_cores=list(range(N_CORES)), stitch_traces=False)

    res = bass_utils.run_bass_kernel_spmd(
        nc, in_maps, core_ids=list(range(N_CORES)), **kwargs
    )

    S = np.zeros(N, dtype=np.float64)
    cg = np.zeros(N, dtype=np.float64)
    for k in range(N_CORES):
        S += res.results[k]["s_out"].astype(np.float64).T.reshape(N)
        gvals = res.results[k]["g_out"].astype(np.float64).T.reshape(K)
        owned = owned_lists[k]
        cg[owned] = gvals[: len(owned)]

    lse = STAB + np.log(
        S - np.exp(SCALE * cg - STAB) + np.exp(SCALE * cg - SCALE * MARGIN - STAB)
    )
    nll = lse - (SCALE * cg - SCALE * MARGIN)
    loss = np.float32(nll.mean())
    return loss, res


def kernel(embedding, ground_truth, weight):
    loss, _ = run(embedding, ground_truth, weight, trace=False)
    return np.asarray(loss, dtype=np.float32)
